# revision 20
# baseline (speedup 1.0000x reference)
"""Trainium2 Bass kernel for nn_MultiHeadSelector.

Data-parallel over batch: 32 samples -> 8 cores x 4 samples.
Per sample:
  score = x[:, :, 0, 1:]                       [12, 784]
  per-head top-84 membership -> histogram counts (match_replace trick)
  3x3 [1 2 1; 2 4 2; 1 2 1] conv on 28x28 grid (shifted adds + boundary fixups)
  stable descending argsort via key = cnt*1024 + (1023 - j), ordered top-84
    (11 rounds of DVE max8/max_index/match_replace)
  GCN, using that only row `basic` of g2 is ever read:
    add = lrelu(pos_w[r] @ (relu(pos_w @ (struct@w1)) @ w2))
        = lrelu(((pos_w[r] @ G1) @ w2)          (associativity saves S2)
    G1[n, h] = relu(pos_w @ S1) in natural layout so the row-dot
    contraction runs on PE; pos_wT built by PE block transposes of the
    DVE-accumulated sum over heads.
  hs = hidden_states with row0 += add;  selected = gather(hs, patch_idx).
"""

import os

import numpy as np

import concourse.bass as bass
import concourse.bacc as bacc
import concourse.mybir as mybir
from concourse.tile import TileContext
from concourse.bass_utils import run_bass_kernel_spmd
from concourse.masks import make_identity

F32 = mybir.dt.float32
U32 = mybir.dt.uint32
I32 = mybir.dt.int32

B, C, SP, S, HID = 32, 12, 785, 784, 768
NCORES = 8
BPC = B // NCORES           # samples per core
K = 84
H1 = 512                    # w1 output dim
GRID = 28
NT = 7                      # 784 / 112 partition tiles
PT = 112
NEG = -1.0e30
POS = 1.0e30

LAST_RESULT = None          # BassKernelResults of the most recent run


def _sample_stream(nc, tc, x3, b, pools, consts):
    """Stream x[b] in [128, 785] row tiles (full 16-port DMA), accumulate the
    head sum on DVE, PE-transpose [rows, 112] blocks into pos_wT m-tiles,
    and compute S1 = struct @ (w1/12)."""
    accp, xinp, poswtp, gcnp, psp, smallb = pools
    ident = consts["ident"]
    w1_sb = consts["w1_sb"]
    starts = [1 + 128 * i for i in range(7)]
    sizes = [128] * 6 + [16]

    scoreb = smallb.tile([C, S], F32, tag="scoreb")
    nc.sync.dma_start(scoreb[:], x3[b * C:(b + 1) * C, 0, 1:])
    s1 = [gcnp.tile([PT, H1], F32, tag=f"s1_{t}", name=f"s1_{t}")
          for t in range(NT)]
    for t in range(NT):
        ps = psp.tile([PT, H1], F32, tag="b1", bufs=3, space="PSUM", name="s1ps")
        nc.tensor.matmul(ps[:], lhsT=scoreb[:, t * PT:(t + 1) * PT],
                         rhs=w1_sb[:], start=True, stop=True)
        nc.scalar.copy(s1[t][:], ps[:])

    acc = [accp.tile([sizes[i], SP], F32, tag=f"acc{i}", name=f"acc{i}")
           for i in range(7)]
    poswt = [poswtp.tile([PT, S], F32, tag=f"pw{t}", name=f"pw{t}")
             for t in range(NT)]
    for i in range(7):
        r0, sz = starts[i], sizes[i]
        nc.sync.dma_start(acc[i][:], x3[b * C, r0:r0 + sz, :])
        eng = nc.gpsimd if i in (4, 6) else nc.vector
        for c in range(1, C):
            xt = xinp.tile([sizes[i], SP], F32, tag="xt", name="xt")
            nc.sync.dma_start(xt[:sz, :], x3[b * C + c, r0:r0 + sz, :])
            eng.tensor_add(acc[i][:], acc[i][:], xt[:sz, :])
        # transpose [sz, 112] blocks into pos_wT m-tiles
        for mt in range(NT):
            pst = psp.tile([PT, 128], F32, tag="b1", bufs=3, space="PSUM",
                           name="tp")
            nc.tensor.transpose(pst[:, 0:sz],
                                acc[i][:, 1 + mt * PT:1 + (mt + 1) * PT],
                                ident[0:sz, 0:sz])
            nc.scalar.copy(poswt[mt][:, r0 - 1:r0 - 1 + sz], pst[:, 0:sz])
    return s1, poswt


def _sample_gcn(nc, tc, x2d, hid2d, hs_out, b, s1, poswt, pools, consts):
    """G1T[h, n] = relu(S1.T @ pos_wT); row-dot v = pos_w[r] @ G1 done as a
    DVE broadcast-multiply-reduce over G1T; add = lrelu(v @ w2)."""
    accp, xinp, poswtp, gcnp, psp, smallb = pools
    ones12 = consts["ones12"]
    ones1x128 = consts["ones1x128"]
    w2_sb = consts["w2_sb"]
    gidx_u = consts["gidx_u"]

    g1t = [gcnp.tile([128, S], F32, tag=f"g1t_{h}", name=f"g1t_{h}")
           for h in range(4)]
    for h in range(4):
        for c0, c1 in ((0, 512), (512, S)):
            ps = psp.tile([128, 512], F32, tag="b1", bufs=3, space="PSUM",
                          name="g1ps")
            for kt in range(NT):
                nc.tensor.matmul(ps[:, 0:c1 - c0],
                                 lhsT=s1[kt][:, h * 128:(h + 1) * 128],
                                 rhs=poswt[kt][:, c0:c1],
                                 start=(kt == 0), stop=(kt == NT - 1))
            nc.scalar.activation(g1t[h][:, c0:c1], ps[:, 0:c1 - c0],
                                 mybir.ActivationFunctionType.Relu)

    # pos_w row `basic`: gather 12 rows of x, reduce over heads, scale 1/12
    xrow = smallb.tile([C, SP], F32, tag="xrow")
    nc.gpsimd.indirect_dma_start(
        out=xrow[:], out_offset=None, in_=x2d[:],
        in_offset=bass.IndirectOffsetOnAxis(ap=gidx_u[:, b:b + 1], axis=0),
    )
    ps_row = psp.tile([1, S], F32, tag="b2", bufs=2, space="PSUM", name="prowps")
    nc.tensor.matmul(ps_row[:, 0:512], lhsT=ones12[:],
                     rhs=xrow[:, 1:513], start=True, stop=True)
    nc.tensor.matmul(ps_row[:, 512:S], lhsT=ones12[:],
                     rhs=xrow[:, 513:SP], start=True, stop=True)
    prow = smallb.tile([1, S], F32, tag="prow_sb", bufs=1)
    nc.scalar.mul(prow[:], ps_row[:], 1.0 / 12.0)

    # broadcast prow to 128 partitions, then v[h] = sum_n g1t[h]*prow_bc
    psb = psp.tile([128, S], F32, tag="b2", bufs=2, space="PSUM", name="psb")
    nc.tensor.matmul(psb[:, 0:512], lhsT=ones1x128[:], rhs=prow[0:1, 0:512],
                     start=True, stop=True)
    nc.tensor.matmul(psb[:, 512:S], lhsT=ones1x128[:], rhs=prow[0:1, 512:S],
                     start=True, stop=True)
    prow_bc = smallb.tile([128, S], F32, tag="prowbc", bufs=1)
    nc.scalar.copy(prow_bc[:], psb[:])
    vcol = smallb.tile([128, 4], F32, tag="vcol")
    vscr = smallb.tile([128, S], F32, tag="vscr", bufs=1)
    for h in range(4):
        nc.vector.tensor_mul(vscr[:], g1t[h][:], prow_bc[:])
        nc.vector.reduce_sum(vcol[:, h:h + 1], vscr[:],
                             axis=mybir.AxisListType.X)

    # add = lrelu(v @ w2) -> [1, 768]
    ps_add = psp.tile([1, HID], F32, tag="b2", bufs=2, space="PSUM", name="addps")
    for c0, c1 in ((0, 512), (512, HID)):
        for kt in range(4):
            nc.tensor.matmul(ps_add[:, c0:c1], lhsT=vcol[:, kt:kt + 1],
                             rhs=w2_sb[kt][:, c0:c1],
                             start=(kt == 0), stop=(kt == 3))
    # lrelu(v) = max(v, 0.2*v)
    t1 = smallb.tile([1, HID], F32, tag="lr1", bufs=1)
    nc.scalar.copy(t1[:], ps_add[:])
    t2 = smallb.tile([1, HID], F32, tag="lr2", bufs=1)
    nc.vector.tensor_scalar_mul(t2[:], t1[:], 0.2)
    addr = smallb.tile([1, HID], F32, tag="addrow", bufs=1)
    nc.vector.tensor_max(addr[:], t1[:], t2[:])
    hs0 = smallb.tile([1, HID], F32, tag="hs0", bufs=1)
    nc.scalar.dma_start(hs0[:], hid2d[b * SP:b * SP + 1, :])
    nc.vector.tensor_add(hs0[:], hs0[:], addr[:])
    nc.scalar.dma_start(hs_out[b * SP:b * SP + 1, :], hs0[:])


def _phase_a_early(nc, tc, psp, consts):
    """meanscore -> basic_index -> gidx_u (row ids for every sample's pos_w
    row gather). Cheap; emitted early so GCNs are never blocked on it."""
    ident = consts["ident"]
    gbase_sb = consts["gbase_sb"]
    bsel_sb = consts["bsel_sb"]
    gidx_u = consts["gidx_u"]
    score_sb = consts["score_sb"]

    with tc.tile_pool(name="earlyA", bufs=1) as smalla:
        ps_msc = psp.tile([BPC, S], F32, tag="b2", bufs=2, space="PSUM",
                          name="ps_msc")
        for c0, c1 in ((0, 512), (512, S)):
            nc.tensor.matmul(ps_msc[:, c0:c1], lhsT=bsel_sb[:],
                             rhs=score_sb[:, c0:c1], start=True, stop=True)
        meansc = smalla.tile([BPC, S], F32, tag="meansc")
        nc.scalar.copy(meansc[:], ps_msc[:])

        msx = smalla.tile([BPC, 8], F32, tag="msx")
        nc.vector.max(msx[:], meansc[:])
        msi = smalla.tile([BPC, 8], U32, tag="msi")
        nc.vector.max_index(msi[:], msx[:], meansc[:])
        rf = smalla.tile([BPC, 1], F32, tag="rf")
        nc.vector.tensor_copy(rf[:], msi[:, 0:1])
        ps_rT = psp.tile([1, BPC], F32, tag="b1", bufs=3, space="PSUM",
                         name="ps_rT")
        nc.tensor.transpose(ps_rT[:], rf[:], ident[0:BPC, 0:BPC])
        rf_row = smalla.tile([1, BPC], F32, tag="rfrow")
        nc.scalar.copy(rf_row[:], ps_rT[:])
        ps_rbc = psp.tile([C, BPC], F32, tag="b1", bufs=3, space="PSUM",
                          name="ps_rbc")
        nc.tensor.matmul(ps_rbc[:], lhsT=consts["ones1x12"][:], rhs=rf_row[:],
                         start=True, stop=True)
        gidx_f = smalla.tile([C, BPC], F32, tag="gidxf")
        nc.vector.tensor_add(gidx_f[:], gbase_sb[:], ps_rbc[:])
        nc.vector.tensor_copy(gidx_u[:], gidx_f[:])


def _phase_a_late(nc, tc, psp, hid2d, sel_out, pidx_out, consts):
    """Per-head top-84 -> histogram counts -> conv -> stable sort ->
    patch_idx + selected gather. Pure output path; emitted last so the DVE
    sort runs under the PE-bound tail instead of stalling the stream."""
    ident = consts["ident"]
    selbase_sb = consts["selbase_sb"]
    bsel_sb = consts["bsel_sb"]
    iota_rev = consts["iota_rev"]
    score_sb = consts["score_sb"]

    with (
        tc.tile_pool(name="sortA", bufs=1) as sortp,
        tc.tile_pool(name="smallA", bufs=1) as smalla,
        tc.tile_pool(name="gathA", bufs=1) as gatha,
    ):
        # per-head ordered top-84 membership via match_replace
        wk = [sortp.tile([BPC * C, S], F32, tag="pp", bufs=2, name=f"wk{i}")
              for i in range(2)]
        nc.vector.tensor_copy(wk[0][:], score_sb[:])
        cur = 0
        for it in range(11):
            mx = sortp.tile([BPC * C, 8], F32, tag="mx8", bufs=2)
            nc.vector.max(mx[:], wk[cur][:])
            if it == 10:
                nc.vector.memset(mx[:, 4:8], POS)
            nc.vector.match_replace(wk[1 - cur][:], mx[:], wk[cur][:], NEG)
            cur = 1 - cur
        eqm = sortp.tile([BPC * C, S], F32, tag="eqm")
        nc.vector.tensor_scalar(eqm[:], wk[cur][:], NEG, None,
                                mybir.AluOpType.is_equal)

        ps_cnt = psp.tile([BPC, S], F32, tag="b2", bufs=2, space="PSUM",
                          name="ps_cnt")
        for c0, c1 in ((0, 512), (512, S)):
            nc.tensor.matmul(ps_cnt[:, c0:c1], lhsT=bsel_sb[:], rhs=eqm[:, c0:c1],
                             start=True, stop=True)
        counts = smalla.tile([BPC, S], F32, tag="counts")
        nc.scalar.copy(counts[:], ps_cnt[:])

        # 3x3 separable conv on the 28x28 grid
        hh = smalla.tile([BPC, S], F32, tag="hh")
        nc.vector.tensor_scalar_mul(hh[:], counts[:], 2.0)
        nc.vector.tensor_add(hh[:, 1:S], hh[:, 1:S], counts[:, 0:S - 1])
        nc.vector.tensor_add(hh[:, 0:S - 1], hh[:, 0:S - 1], counts[:, 1:S])
        # undo wrap-around across 28-col row boundaries (27 positions each)
        hh3a = hh[:, GRID:S].rearrange("p (r c) -> p r c", c=GRID)[:, :, 0:1]
        cn3a = counts[:, GRID - 1:S - 1].rearrange("p (r c) -> p r c", c=GRID)[:, :, 0:1]
        nc.vector.tensor_sub(hh3a, hh3a, cn3a)
        hh3b = hh[:, GRID - 1:S - 1].rearrange("p (r c) -> p r c", c=GRID)[:, :, 0:1]
        cn3b = counts[:, GRID:S].rearrange("p (r c) -> p r c", c=GRID)[:, :, 0:1]
        nc.vector.tensor_sub(hh3b, hh3b, cn3b)
        vv = smalla.tile([BPC, S], F32, tag="vv")
        nc.vector.tensor_scalar_mul(vv[:], hh[:], 2.0)
        nc.vector.tensor_add(vv[:, GRID:S], vv[:, GRID:S], hh[:, 0:S - GRID])
        nc.vector.tensor_add(vv[:, 0:S - GRID], vv[:, 0:S - GRID], hh[:, GRID:S])

        # sort key, ordered top-84
        key = [sortp.tile([BPC * C, S], F32, tag="pp", bufs=2,
                          name=f"key{i}")[0:BPC, :]
               for i in range(2)]
        nc.vector.tensor_scalar_mul(key[0][:], vv[:], 1024.0)
        nc.vector.tensor_add(key[0][:], key[0][:], iota_rev[:])
        mxi = smalla.tile([BPC, 88], U32, tag="mxi")
        cur = 0
        for it in range(11):
            mxk = sortp.tile([BPC, 8], F32, tag="mxk", bufs=2)
            nc.vector.max(mxk[:], key[cur][:])
            nc.vector.max_index(mxi[:, it * 8:it * 8 + 8], mxk[:], key[cur][:])
            if it < 10:
                nc.vector.match_replace(key[1 - cur][:], mxk[:], key[cur][:], NEG)
                cur = 1 - cur

        pidx_f = smalla.tile([BPC, K], F32, tag="pidxf")
        nc.vector.tensor_copy(pidx_f[:], mxi[:, 0:K])
        nc.vector.tensor_scalar_add(pidx_f[:], pidx_f[:], 1.0)
        pidx_i = smalla.tile([BPC, K], I32, tag="pidxi")
        nc.vector.tensor_copy(pidx_i[:], pidx_f[:])
        nc.scalar.dma_start(pidx_out[:], pidx_i[:])

        # selected gather: row ids b*785 + patch_idx, transposed to [84, BPC]
        grow_f = smalla.tile([BPC, K], F32, tag="growf")
        nc.vector.tensor_add(grow_f[:], pidx_f[:],
                             selbase_sb[:, 0:1].to_broadcast([BPC, K]))
        ps_gT = psp.tile([K, BPC], F32, tag="b1", bufs=3, space="PSUM",
                         name="ps_gT")
        nc.tensor.transpose(ps_gT[:], grow_f[:], ident[0:BPC, 0:BPC])
        growT_f = smalla.tile([K, BPC], F32, tag="growtf")
        nc.scalar.copy(growT_f[:], ps_gT[:])
        growT_u = smalla.tile([K, BPC], U32, tag="growtu")
        nc.vector.tensor_copy(growT_u[:], growT_f[:])

        for b in range(BPC):
            g = gatha.tile([K, HID], F32, tag="gath")
            nc.gpsimd.indirect_dma_start(
                out=g[:], out_offset=None, in_=hid2d[:],
                in_offset=bass.IndirectOffsetOnAxis(ap=growT_u[:, b:b + 1], axis=0),
            )
            nc.scalar.dma_start(sel_out[b * K:(b + 1) * K, :], g[:])


def _build_kernel():
    nc = bacc.Bacc("TRN2", target_bir_lowering=False, debug=False,
                   num_devices=NCORES)

    x2d = nc.dram_tensor("x", [BPC * C * SP, SP], F32, kind="ExternalInput").ap()
    hid2d = nc.dram_tensor("hid", [BPC * SP, HID], F32, kind="ExternalInput").ap()
    w1d = nc.dram_tensor("w1", [C, H1], F32, kind="ExternalInput").ap()
    w2d = nc.dram_tensor("w2", [H1, HID], F32, kind="ExternalInput").ap()
    gbased = nc.dram_tensor("gbase", [C, BPC], F32, kind="ExternalInput").ap()
    selbased = nc.dram_tensor("selbase", [BPC, 1], F32, kind="ExternalInput").ap()
    bseld = nc.dram_tensor("bsel", [C * BPC, BPC], F32, kind="ExternalInput").ap()

    hs_out = nc.dram_tensor("hs_out", [BPC * SP, HID], F32, kind="ExternalOutput").ap()
    sel_out = nc.dram_tensor("sel_out", [BPC * K, HID], F32, kind="ExternalOutput").ap()
    pidx_out = nc.dram_tensor("pidx_out", [BPC, K], I32, kind="ExternalOutput").ap()

    x3 = x2d.rearrange("(n s) t -> n s t", s=SP)          # [BPC*C, 785, 785]

    with TileContext(nc) as tc:
        with (
            tc.tile_pool(name="const", bufs=1) as constp,
            tc.tile_pool(name="ps", bufs=1, space="PSUM") as psp,
            tc.tile_pool(name="acc", bufs=2) as accp,
            tc.tile_pool(name="xin", bufs=4) as xinp,
            tc.tile_pool(name="poswt", bufs=2) as poswtp,
            tc.tile_pool(name="gcn", bufs=1) as gcnp,
            tc.tile_pool(name="smallB", bufs=2) as smallb,
        ):
            # ---------------- constants ----------------
            ident = constp.tile([128, 128], F32)
            make_identity(nc, ident[:])
            ones12 = constp.tile([C, 1], F32)
            nc.gpsimd.memset(ones12[:], 1.0)
            ones1x12 = constp.tile([1, C], F32)
            nc.gpsimd.memset(ones1x12[:], 1.0)
            ones1x128 = constp.tile([1, 128], F32)
            nc.gpsimd.memset(ones1x128[:], 1.0)
            iota_rev = constp.tile([BPC, S], F32)
            nc.gpsimd.iota(iota_rev[:], pattern=[[-1, S]], base=1023,
                           channel_multiplier=0,
                           allow_small_or_imprecise_dtypes=True)

            w1_sb = constp.tile([C, H1], F32)
            nc.sync.dma_start(w1_sb[:], w1d[:])
            w2_sb = [constp.tile([128, HID], F32, tag=f"w2_{k}", name=f"w2sb{k}")
                     for k in range(4)]
            for k in range(4):
                nc.scalar.dma_start(w2_sb[k][:], w2d[k * 128:(k + 1) * 128, :])
            gbase_sb = constp.tile([C, BPC], F32)
            nc.scalar.dma_start(gbase_sb[:], gbased[:])
            selbase_sb = constp.tile([BPC, 1], F32)
            nc.scalar.dma_start(selbase_sb[:], selbased[:])
            bsel_sb = constp.tile([C * BPC, BPC], F32)
            nc.scalar.dma_start(bsel_sb[:], bseld[:])
            gidx_u = constp.tile([C, BPC], U32)
            score_sb = constp.tile([BPC * C, S], F32)
            nc.sync.dma_start(score_sb[:], x3[:, 0, 1:])

            consts = dict(ident=ident, ones12=ones12, ones1x12=ones1x12,
                          ones1x128=ones1x128,
                          iota_rev=iota_rev, gbase_sb=gbase_sb,
                          selbase_sb=selbase_sb, bsel_sb=bsel_sb,
                          gidx_u=gidx_u, w1_sb=w1_sb, w2_sb=w2_sb,
                          score_sb=score_sb)
            pools = (accp, xinp, poswtp, gcnp, psp, smallb)

            # sample 0 streaming first so DMA/DVE start immediately
            s1_0, poswt_0 = _sample_stream(nc, tc, x3, 0, pools, consts)
            # basic_index/gidx_u only (cheap) — the heavy sort is deferred
            _phase_a_early(nc, tc, psp, consts)
            _sample_gcn(nc, tc, x2d, hid2d, hs_out, 0, s1_0, poswt_0,
                        pools, consts)

            for b in range(1, BPC):
                s1_b, poswt_b = _sample_stream(nc, tc, x3, b, pools, consts)
                if b == BPC - 1:
                    # heavy sort + outputs run under the PE-bound tail
                    _phase_a_late(nc, tc, psp, hid2d, sel_out, pidx_out,
                                  consts)
                _sample_gcn(nc, tc, x2d, hid2d, hs_out, b, s1_b, poswt_b,
                            pools, consts)

            # bulk hs copy (rows 1..784), DRAM->DRAM on the scalar queue,
            # emitted last so it fills the PE-bound tail
            for b in range(BPC):
                nc.scalar.dma_start(hs_out[b * SP + 1:(b + 1) * SP, :],
                                    hid2d[b * SP + 1:(b + 1) * SP, :])

    nc.compile()
    return nc


def _make_inputs(inputs):
    x = np.ascontiguousarray(inputs["x"], dtype=np.float32)
    hid = np.ascontiguousarray(inputs["hidden_states"], dtype=np.float32)
    # fold the pos_w 1/12 head-mean into w1 (the row-dot path scales prow)
    w1 = np.ascontiguousarray(inputs["w1"], dtype=np.float32) / 12.0
    w2 = np.ascontiguousarray(inputs["w2"], dtype=np.float32)

    gbase = np.empty((C, BPC), np.float32)
    for c in range(C):
        for b in range(BPC):
            gbase[c, b] = (b * C + c) * SP + 1
    selbase = (np.arange(BPC, dtype=np.float32) * SP).reshape(BPC, 1)
    bsel = np.zeros((C * BPC, BPC), np.float32)
    for b in range(BPC):
        bsel[b * C:(b + 1) * C, b] = 1.0

    in_maps = []
    for core in range(NCORES):
        b0 = core * BPC
        in_maps.append({
            "x": x[b0:b0 + BPC].reshape(BPC * C * SP, SP),
            "hid": hid[b0:b0 + BPC].reshape(BPC * SP, HID),
            "w1": w1, "w2": w2,
            "gbase": gbase, "selbase": selbase, "bsel": bsel,
        })
    return in_maps


def kernel(hidden_states, x, contribution, w1, w2):
    global LAST_RESULT
    nc = _build_kernel()
    in_maps = _make_inputs({"hidden_states": hidden_states, "x": x,
                            "w1": w1, "w2": w2})
    tmpdir = os.environ.get("BASS_KERNEL_TMPDIR")
    if tmpdir:
        os.makedirs(tmpdir, exist_ok=True)
    res = run_bass_kernel_spmd(nc, in_maps, list(range(NCORES)), tmpdir=tmpdir)
    LAST_RESULT = res
    hs = np.concatenate([r["hs_out"].reshape(BPC, SP, HID) for r in res.results])
    sel = np.concatenate([r["sel_out"].reshape(BPC, K, HID) for r in res.results])
    pidx = np.concatenate([r["pidx_out"].reshape(BPC, K) for r in res.results])
    return hs, sel.astype(np.float32), pidx.astype(np.int32)


# revision 22
# speedup vs baseline: 1.0578x; 1.0578x over previous
"""Trainium2 Bass kernel for nn_MultiHeadSelector.

Data-parallel over batch: 32 samples -> 8 cores x 4 samples.
Per sample:
  score = x[:, :, 0, 1:]                       [12, 784]
  per-head top-84 membership -> histogram counts (match_replace trick)
  3x3 [1 2 1; 2 4 2; 1 2 1] conv on 28x28 grid (shifted adds + boundary fixups)
  stable descending argsort via key = cnt*1024 + (1023 - j), ordered top-84
    (11 rounds of DVE max8/max_index/match_replace)
  GCN, using that only row `basic` of g2 is ever read:
    add = lrelu(pos_w[r] @ (relu(pos_w @ (struct@w1)) @ w2))
        = lrelu(((pos_w[r] @ G1) @ w2)          (associativity saves S2)
    G1[n, h] = relu(pos_w @ S1) in natural layout so the row-dot
    contraction runs on PE; pos_wT built by PE block transposes of the
    DVE-accumulated sum over heads.
  hs = hidden_states with row0 += add;  selected = gather(hs, patch_idx).
"""

import os

import numpy as np

import concourse.bass as bass
import concourse.bacc as bacc
import concourse.mybir as mybir
from concourse.tile import TileContext
from concourse.bass_utils import run_bass_kernel_spmd
from concourse.masks import make_identity

F32 = mybir.dt.float32
U32 = mybir.dt.uint32
I32 = mybir.dt.int32

B, C, SP, S, HID = 32, 12, 785, 784, 768
NCORES = 8
BPC = B // NCORES           # samples per core
K = 84
H1 = 512                    # w1 output dim
GRID = 28
NT = 7                      # 784 / 112 partition tiles
PT = 112
NEG = -1.0e30
POS = 1.0e30

LAST_RESULT = None          # BassKernelResults of the most recent run


def _sample_stream(nc, tc, x3, b, pools, consts):
    """Stream x[b] in [128, 785] row tiles (full 16-port DMA), accumulate the
    head sum on DVE, PE-transpose [rows, 112] blocks into pos_wT m-tiles,
    and compute S1 = struct @ (w1/12)."""
    accp, xinp, poswtp, gcnp, psp, smallb = pools
    ident = consts["ident"]
    w1_sb = consts["w1_sb"]
    starts = [1 + 128 * i for i in range(7)]
    sizes = [128] * 6 + [16]

    scoreb = smallb.tile([C, S], F32, tag="scoreb")
    nc.sync.dma_start(scoreb[:], x3[b * C:(b + 1) * C, 0, 1:])
    s1 = [gcnp.tile([PT, H1], F32, tag=f"s1_{t}", name=f"s1_{t}")
          for t in range(NT)]
    for t in range(NT):
        ps = psp.tile([PT, H1], F32, tag="b1", bufs=3, space="PSUM", name="s1ps")
        nc.tensor.matmul(ps[:], lhsT=scoreb[:, t * PT:(t + 1) * PT],
                         rhs=w1_sb[:], start=True, stop=True)
        nc.scalar.copy(s1[t][:], ps[:])

    acc = [accp.tile([sizes[i], SP], F32, tag=f"acc{i}", name=f"acc{i}")
           for i in range(7)]
    poswt = [poswtp.tile([PT, S], F32, tag=f"pw{t}", name=f"pw{t}")
             for t in range(NT)]
    for i in range(7):
        r0, sz = starts[i], sizes[i]
        nc.sync.dma_start(acc[i][:], x3[b * C, r0:r0 + sz, :])
        for c in range(1, C):
            xt = xinp.tile([sizes[i], SP], F32, tag="xt", name="xt")
            nc.sync.dma_start(xt[:sz, :], x3[b * C + c, r0:r0 + sz, :])
            nc.vector.tensor_add(acc[i][:], acc[i][:], xt[:sz, :])
        # transpose [sz, 112] blocks into pos_wT m-tiles
        for mt in range(NT):
            pst = psp.tile([PT, 128], F32, tag="b1", bufs=3, space="PSUM",
                           name="tp")
            nc.tensor.transpose(pst[:, 0:sz],
                                acc[i][:, 1 + mt * PT:1 + (mt + 1) * PT],
                                ident[0:sz, 0:sz])
            nc.scalar.copy(poswt[mt][:, r0 - 1:r0 - 1 + sz], pst[:, 0:sz])
    return s1, poswt


def _sample_gcn(nc, tc, x2d, hid2d, hs_out, b, s1, poswt, pools, consts):
    """G1T[h, n] = relu(S1.T @ pos_wT); row-dot v = pos_w[r] @ G1 done as a
    DVE broadcast-multiply-reduce over G1T; add = lrelu(v @ w2)."""
    accp, xinp, poswtp, gcnp, psp, smallb = pools
    ones12 = consts["ones12"]
    ones1x128 = consts["ones1x128"]
    w2_sb = consts["w2_sb"]
    gidx_u = consts["gidx_u"]

    g1t = [gcnp.tile([128, S], F32, tag=f"g1t_{h}", name=f"g1t_{h}")
           for h in range(4)]
    for h in range(4):
        for c0, c1 in ((0, 512), (512, S)):
            ps = psp.tile([128, 512], F32, tag="b1", bufs=3, space="PSUM",
                          name="g1ps")
            for kt in range(NT):
                nc.tensor.matmul(ps[:, 0:c1 - c0],
                                 lhsT=s1[kt][:, h * 128:(h + 1) * 128],
                                 rhs=poswt[kt][:, c0:c1],
                                 start=(kt == 0), stop=(kt == NT - 1))
            nc.scalar.activation(g1t[h][:, c0:c1], ps[:, 0:c1 - c0],
                                 mybir.ActivationFunctionType.Relu)

    # pos_w row `basic`: gather 12 rows of x, reduce over heads, scale 1/12
    xrow = smallb.tile([C, SP], F32, tag="xrow")
    nc.gpsimd.indirect_dma_start(
        out=xrow[:], out_offset=None, in_=x2d[:],
        in_offset=bass.IndirectOffsetOnAxis(ap=gidx_u[:, b:b + 1], axis=0),
    )
    ps_row = psp.tile([1, S], F32, tag="b2", bufs=2, space="PSUM", name="prowps")
    nc.tensor.matmul(ps_row[:, 0:512], lhsT=ones12[:],
                     rhs=xrow[:, 1:513], start=True, stop=True)
    nc.tensor.matmul(ps_row[:, 512:S], lhsT=ones12[:],
                     rhs=xrow[:, 513:SP], start=True, stop=True)
    prow = smallb.tile([1, S], F32, tag="prow_sb", bufs=1)
    nc.scalar.mul(prow[:], ps_row[:], 1.0 / 12.0)

    # broadcast prow to 128 partitions, then v[h] = sum_n g1t[h]*prow_bc
    psb = psp.tile([128, S], F32, tag="b2", bufs=2, space="PSUM", name="psb")
    nc.tensor.matmul(psb[:, 0:512], lhsT=ones1x128[:], rhs=prow[0:1, 0:512],
                     start=True, stop=True)
    nc.tensor.matmul(psb[:, 512:S], lhsT=ones1x128[:], rhs=prow[0:1, 512:S],
                     start=True, stop=True)
    prow_bc = smallb.tile([128, S], F32, tag="prowbc", bufs=1)
    nc.scalar.copy(prow_bc[:], psb[:])
    vcol = smallb.tile([128, 4], F32, tag="vcol")
    vscr = smallb.tile([128, S], F32, tag="vscr", bufs=1)
    for h in range(4):
        nc.vector.tensor_mul(vscr[:], g1t[h][:], prow_bc[:])
        nc.vector.reduce_sum(vcol[:, h:h + 1], vscr[:],
                             axis=mybir.AxisListType.X)

    # add = lrelu(v @ w2) -> [1, 768]
    ps_add = psp.tile([1, HID], F32, tag="b2", bufs=2, space="PSUM", name="addps")
    for c0, c1 in ((0, 512), (512, HID)):
        for kt in range(4):
            nc.tensor.matmul(ps_add[:, c0:c1], lhsT=vcol[:, kt:kt + 1],
                             rhs=w2_sb[kt][:, c0:c1],
                             start=(kt == 0), stop=(kt == 3))
    # lrelu(v) = max(v, 0.2*v)
    t1 = smallb.tile([1, HID], F32, tag="lr1", bufs=1)
    nc.scalar.copy(t1[:], ps_add[:])
    t2 = smallb.tile([1, HID], F32, tag="lr2", bufs=1)
    nc.vector.tensor_scalar_mul(t2[:], t1[:], 0.2)
    addr = smallb.tile([1, HID], F32, tag="addrow", bufs=1)
    nc.vector.tensor_max(addr[:], t1[:], t2[:])
    hs0 = smallb.tile([1, HID], F32, tag="hs0", bufs=1)
    nc.scalar.dma_start(hs0[:], hid2d[b * SP:b * SP + 1, :])
    nc.vector.tensor_add(hs0[:], hs0[:], addr[:])
    nc.scalar.dma_start(hs_out[b * SP:b * SP + 1, :], hs0[:])


def _phase_a_early(nc, tc, psp, consts):
    """meanscore -> basic_index -> gidx_u (row ids for every sample's pos_w
    row gather). Cheap; emitted early so GCNs are never blocked on it."""
    ident = consts["ident"]
    gbase_sb = consts["gbase_sb"]
    bsel_sb = consts["bsel_sb"]
    gidx_u = consts["gidx_u"]
    score_sb = consts["score_sb"]

    with tc.tile_pool(name="earlyA", bufs=1) as smalla:
        ps_msc = psp.tile([BPC, S], F32, tag="b2", bufs=2, space="PSUM",
                          name="ps_msc")
        for c0, c1 in ((0, 512), (512, S)):
            nc.tensor.matmul(ps_msc[:, c0:c1], lhsT=bsel_sb[:],
                             rhs=score_sb[:, c0:c1], start=True, stop=True)
        meansc = smalla.tile([BPC, S], F32, tag="meansc")
        nc.scalar.copy(meansc[:], ps_msc[:])

        msx = smalla.tile([BPC, 8], F32, tag="msx")
        nc.vector.max(msx[:], meansc[:])
        msi = smalla.tile([BPC, 8], U32, tag="msi")
        nc.vector.max_index(msi[:], msx[:], meansc[:])
        rf = smalla.tile([BPC, 1], F32, tag="rf")
        nc.vector.tensor_copy(rf[:], msi[:, 0:1])
        ps_rT = psp.tile([1, BPC], F32, tag="b1", bufs=3, space="PSUM",
                         name="ps_rT")
        nc.tensor.transpose(ps_rT[:], rf[:], ident[0:BPC, 0:BPC])
        rf_row = smalla.tile([1, BPC], F32, tag="rfrow")
        nc.scalar.copy(rf_row[:], ps_rT[:])
        ps_rbc = psp.tile([C, BPC], F32, tag="b1", bufs=3, space="PSUM",
                          name="ps_rbc")
        nc.tensor.matmul(ps_rbc[:], lhsT=consts["ones1x12"][:], rhs=rf_row[:],
                         start=True, stop=True)
        gidx_f = smalla.tile([C, BPC], F32, tag="gidxf")
        nc.vector.tensor_add(gidx_f[:], gbase_sb[:], ps_rbc[:])
        nc.vector.tensor_copy(gidx_u[:], gidx_f[:])


def _phase_a_late(nc, tc, psp, hid2d, sel_out, pidx_out, consts):
    """Per-head top-84 -> histogram counts -> conv -> stable sort ->
    patch_idx + selected gather. Pure output path; emitted last so the DVE
    sort runs under the PE-bound tail instead of stalling the stream."""
    ident = consts["ident"]
    selbase_sb = consts["selbase_sb"]
    bsel_sb = consts["bsel_sb"]
    iota_rev = consts["iota_rev"]
    score_sb = consts["score_sb"]

    with (
        tc.tile_pool(name="sortA", bufs=1) as sortp,
        tc.tile_pool(name="smallA", bufs=1) as smalla,
        tc.tile_pool(name="gathA", bufs=1) as gatha,
    ):
        # per-head ordered top-84 membership via match_replace
        wk = [sortp.tile([BPC * C, S], F32, tag="pp", bufs=2, name=f"wk{i}")
              for i in range(2)]
        nc.vector.tensor_copy(wk[0][:], score_sb[:])
        cur = 0
        for it in range(11):
            mx = sortp.tile([BPC * C, 8], F32, tag="mx8", bufs=2)
            nc.vector.max(mx[:], wk[cur][:])
            if it == 10:
                nc.vector.memset(mx[:, 4:8], POS)
            nc.vector.match_replace(wk[1 - cur][:], mx[:], wk[cur][:], NEG)
            cur = 1 - cur
        eqm = sortp.tile([BPC * C, S], F32, tag="eqm")
        nc.vector.tensor_scalar(eqm[:], wk[cur][:], NEG, None,
                                mybir.AluOpType.is_equal)

        ps_cnt = psp.tile([BPC, S], F32, tag="b2", bufs=2, space="PSUM",
                          name="ps_cnt")
        for c0, c1 in ((0, 512), (512, S)):
            nc.tensor.matmul(ps_cnt[:, c0:c1], lhsT=bsel_sb[:], rhs=eqm[:, c0:c1],
                             start=True, stop=True)
        counts = smalla.tile([BPC, S], F32, tag="counts")
        nc.scalar.copy(counts[:], ps_cnt[:])

        # 3x3 separable conv on the 28x28 grid
        hh = smalla.tile([BPC, S], F32, tag="hh")
        nc.vector.tensor_scalar_mul(hh[:], counts[:], 2.0)
        nc.vector.tensor_add(hh[:, 1:S], hh[:, 1:S], counts[:, 0:S - 1])
        nc.vector.tensor_add(hh[:, 0:S - 1], hh[:, 0:S - 1], counts[:, 1:S])
        # undo wrap-around across 28-col row boundaries (27 positions each)
        hh3a = hh[:, GRID:S].rearrange("p (r c) -> p r c", c=GRID)[:, :, 0:1]
        cn3a = counts[:, GRID - 1:S - 1].rearrange("p (r c) -> p r c", c=GRID)[:, :, 0:1]
        nc.vector.tensor_sub(hh3a, hh3a, cn3a)
        hh3b = hh[:, GRID - 1:S - 1].rearrange("p (r c) -> p r c", c=GRID)[:, :, 0:1]
        cn3b = counts[:, GRID:S].rearrange("p (r c) -> p r c", c=GRID)[:, :, 0:1]
        nc.vector.tensor_sub(hh3b, hh3b, cn3b)
        vv = smalla.tile([BPC, S], F32, tag="vv")
        nc.vector.tensor_scalar_mul(vv[:], hh[:], 2.0)
        nc.vector.tensor_add(vv[:, GRID:S], vv[:, GRID:S], hh[:, 0:S - GRID])
        nc.vector.tensor_add(vv[:, 0:S - GRID], vv[:, 0:S - GRID], hh[:, GRID:S])

        # sort key, ordered top-84
        key = [sortp.tile([BPC * C, S], F32, tag="pp", bufs=2,
                          name=f"key{i}")[0:BPC, :]
               for i in range(2)]
        nc.vector.tensor_scalar_mul(key[0][:], vv[:], 1024.0)
        nc.vector.tensor_add(key[0][:], key[0][:], iota_rev[:])
        mxi = smalla.tile([BPC, 88], U32, tag="mxi")
        cur = 0
        for it in range(11):
            mxk = sortp.tile([BPC, 8], F32, tag="mxk", bufs=2)
            nc.vector.max(mxk[:], key[cur][:])
            nc.vector.max_index(mxi[:, it * 8:it * 8 + 8], mxk[:], key[cur][:])
            if it < 10:
                nc.vector.match_replace(key[1 - cur][:], mxk[:], key[cur][:], NEG)
                cur = 1 - cur

        pidx_f = smalla.tile([BPC, K], F32, tag="pidxf")
        nc.vector.tensor_copy(pidx_f[:], mxi[:, 0:K])
        nc.vector.tensor_scalar_add(pidx_f[:], pidx_f[:], 1.0)
        pidx_i = smalla.tile([BPC, K], I32, tag="pidxi")
        nc.vector.tensor_copy(pidx_i[:], pidx_f[:])
        nc.scalar.dma_start(pidx_out[:], pidx_i[:])

        # selected gather: row ids b*785 + patch_idx, transposed to [84, BPC]
        grow_f = smalla.tile([BPC, K], F32, tag="growf")
        nc.vector.tensor_add(grow_f[:], pidx_f[:],
                             selbase_sb[:, 0:1].to_broadcast([BPC, K]))
        ps_gT = psp.tile([K, BPC], F32, tag="b1", bufs=3, space="PSUM",
                         name="ps_gT")
        nc.tensor.transpose(ps_gT[:], grow_f[:], ident[0:BPC, 0:BPC])
        growT_f = smalla.tile([K, BPC], F32, tag="growtf")
        nc.scalar.copy(growT_f[:], ps_gT[:])
        growT_u = smalla.tile([K, BPC], U32, tag="growtu")
        nc.vector.tensor_copy(growT_u[:], growT_f[:])

        for b in range(BPC):
            g = gatha.tile([K, HID], F32, tag="gath")
            nc.gpsimd.indirect_dma_start(
                out=g[:], out_offset=None, in_=hid2d[:],
                in_offset=bass.IndirectOffsetOnAxis(ap=growT_u[:, b:b + 1], axis=0),
            )
            nc.scalar.dma_start(sel_out[b * K:(b + 1) * K, :], g[:])


def _build_kernel():
    nc = bacc.Bacc("TRN2", target_bir_lowering=False, debug=False,
                   num_devices=NCORES)

    x2d = nc.dram_tensor("x", [BPC * C * SP, SP], F32, kind="ExternalInput").ap()
    hid2d = nc.dram_tensor("hid", [BPC * SP, HID], F32, kind="ExternalInput").ap()
    w1d = nc.dram_tensor("w1", [C, H1], F32, kind="ExternalInput").ap()
    w2d = nc.dram_tensor("w2", [H1, HID], F32, kind="ExternalInput").ap()
    gbased = nc.dram_tensor("gbase", [C, BPC], F32, kind="ExternalInput").ap()
    selbased = nc.dram_tensor("selbase", [BPC, 1], F32, kind="ExternalInput").ap()
    bseld = nc.dram_tensor("bsel", [C * BPC, BPC], F32, kind="ExternalInput").ap()

    hs_out = nc.dram_tensor("hs_out", [BPC * SP, HID], F32, kind="ExternalOutput").ap()
    sel_out = nc.dram_tensor("sel_out", [BPC * K, HID], F32, kind="ExternalOutput").ap()
    pidx_out = nc.dram_tensor("pidx_out", [BPC, K], I32, kind="ExternalOutput").ap()

    x3 = x2d.rearrange("(n s) t -> n s t", s=SP)          # [BPC*C, 785, 785]

    with TileContext(nc) as tc:
        with (
            tc.tile_pool(name="const", bufs=1) as constp,
            tc.tile_pool(name="ps", bufs=1, space="PSUM") as psp,
            tc.tile_pool(name="acc", bufs=2) as accp,
            tc.tile_pool(name="xin", bufs=4) as xinp,
            tc.tile_pool(name="poswt", bufs=2) as poswtp,
            tc.tile_pool(name="gcn", bufs=1) as gcnp,
            tc.tile_pool(name="smallB", bufs=2) as smallb,
        ):
            # ---------------- constants ----------------
            ident = constp.tile([128, 128], F32)
            make_identity(nc, ident[:])
            ones12 = constp.tile([C, 1], F32)
            nc.gpsimd.memset(ones12[:], 1.0)
            ones1x12 = constp.tile([1, C], F32)
            nc.gpsimd.memset(ones1x12[:], 1.0)
            ones1x128 = constp.tile([1, 128], F32)
            nc.gpsimd.memset(ones1x128[:], 1.0)
            iota_rev = constp.tile([BPC, S], F32)
            nc.gpsimd.iota(iota_rev[:], pattern=[[-1, S]], base=1023,
                           channel_multiplier=0,
                           allow_small_or_imprecise_dtypes=True)

            w1_sb = constp.tile([C, H1], F32)
            nc.sync.dma_start(w1_sb[:], w1d[:])
            w2_sb = [constp.tile([128, HID], F32, tag=f"w2_{k}", name=f"w2sb{k}")
                     for k in range(4)]
            for k in range(4):
                nc.scalar.dma_start(w2_sb[k][:], w2d[k * 128:(k + 1) * 128, :])
            gbase_sb = constp.tile([C, BPC], F32)
            nc.scalar.dma_start(gbase_sb[:], gbased[:])
            selbase_sb = constp.tile([BPC, 1], F32)
            nc.scalar.dma_start(selbase_sb[:], selbased[:])
            bsel_sb = constp.tile([C * BPC, BPC], F32)
            nc.scalar.dma_start(bsel_sb[:], bseld[:])
            gidx_u = constp.tile([C, BPC], U32)
            score_sb = constp.tile([BPC * C, S], F32)
            nc.sync.dma_start(score_sb[:], x3[:, 0, 1:])

            consts = dict(ident=ident, ones12=ones12, ones1x12=ones1x12,
                          ones1x128=ones1x128,
                          iota_rev=iota_rev, gbase_sb=gbase_sb,
                          selbase_sb=selbase_sb, bsel_sb=bsel_sb,
                          gidx_u=gidx_u, w1_sb=w1_sb, w2_sb=w2_sb,
                          score_sb=score_sb)
            pools = (accp, xinp, poswtp, gcnp, psp, smallb)

            # sample 0 streaming first so DMA/DVE start immediately
            s1_0, poswt_0 = _sample_stream(nc, tc, x3, 0, pools, consts)
            # basic_index/gidx_u only (cheap) — the heavy sort is deferred
            _phase_a_early(nc, tc, psp, consts)
            _sample_gcn(nc, tc, x2d, hid2d, hs_out, 0, s1_0, poswt_0,
                        pools, consts)

            for b in range(1, BPC):
                s1_b, poswt_b = _sample_stream(nc, tc, x3, b, pools, consts)
                if b == BPC - 1:
                    # heavy sort + outputs run under the PE-bound tail
                    _phase_a_late(nc, tc, psp, hid2d, sel_out, pidx_out,
                                  consts)
                _sample_gcn(nc, tc, x2d, hid2d, hs_out, b, s1_b, poswt_b,
                            pools, consts)

            # bulk hs copy (rows 1..784), DRAM->DRAM on the scalar queue,
            # emitted last so it fills the PE-bound tail
            for b in range(BPC):
                nc.scalar.dma_start(hs_out[b * SP + 1:(b + 1) * SP, :],
                                    hid2d[b * SP + 1:(b + 1) * SP, :])

    nc.compile()
    return nc


def _make_inputs(inputs):
    x = np.ascontiguousarray(inputs["x"], dtype=np.float32)
    hid = np.ascontiguousarray(inputs["hidden_states"], dtype=np.float32)
    # fold the pos_w 1/12 head-mean into w1 (the row-dot path scales prow)
    w1 = np.ascontiguousarray(inputs["w1"], dtype=np.float32) / 12.0
    w2 = np.ascontiguousarray(inputs["w2"], dtype=np.float32)

    gbase = np.empty((C, BPC), np.float32)
    for c in range(C):
        for b in range(BPC):
            gbase[c, b] = (b * C + c) * SP + 1
    selbase = (np.arange(BPC, dtype=np.float32) * SP).reshape(BPC, 1)
    bsel = np.zeros((C * BPC, BPC), np.float32)
    for b in range(BPC):
        bsel[b * C:(b + 1) * C, b] = 1.0

    in_maps = []
    for core in range(NCORES):
        b0 = core * BPC
        in_maps.append({
            "x": x[b0:b0 + BPC].reshape(BPC * C * SP, SP),
            "hid": hid[b0:b0 + BPC].reshape(BPC * SP, HID),
            "w1": w1, "w2": w2,
            "gbase": gbase, "selbase": selbase, "bsel": bsel,
        })
    return in_maps


def _ensure_trace_hooks():
    """If tracing is requested but this container's antenv lacks axon_hooks,
    wire the ctypes NTFF hook from trn_agent_boot and stub the artifact
    upload so run_bass_kernel_spmd's trace path works."""
    import sys
    import types
    try:
        import antenv.axon_hooks  # noqa: F401
        return
    except ImportError:
        pass
    try:
        import antenv
        from trn_agent_boot.trn_boot import _ntff_profile_via_ctypes
        mod = types.ModuleType("antenv.axon_hooks")
        _h = [None]
        mod.set_axon_ntff_profile_hook = lambda h: _h.__setitem__(0, h)
        mod.get_axon_ntff_profile_hook = lambda: _h[0]
        sys.modules["antenv.axon_hooks"] = mod
        antenv.axon_hooks = mod
        hook = _ntff_profile_via_ctypes("/opt/axon/libaxon_pjrt.so")
        if hook is not None:
            mod.set_axon_ntff_profile_hook(hook)
        import concourse.bass_utils as bu
        bu.upload_artifacts = lambda tmpdir: f"file://{tmpdir}"
    except Exception:
        os.environ["BASS_NEVER_TRACE"] = "1"


def kernel(hidden_states, x, contribution, w1, w2):
    global LAST_RESULT
    if os.environ.get("BASS_TRACE"):
        _ensure_trace_hooks()
    nc = _build_kernel()
    in_maps = _make_inputs({"hidden_states": hidden_states, "x": x,
                            "w1": w1, "w2": w2})
    tmpdir = os.environ.get("BASS_KERNEL_TMPDIR")
    if tmpdir:
        os.makedirs(tmpdir, exist_ok=True)
    res = run_bass_kernel_spmd(nc, in_maps, list(range(NCORES)), tmpdir=tmpdir)
    LAST_RESULT = res
    hs = np.concatenate([r["hs_out"].reshape(BPC, SP, HID) for r in res.results])
    sel = np.concatenate([r["sel_out"].reshape(BPC, K, HID) for r in res.results])
    pidx = np.concatenate([r["pidx_out"].reshape(BPC, K) for r in res.results])
    return hs, sel.astype(np.float32), pidx.astype(np.int32)


# revision 23
# speedup vs baseline: 1.2095x; 1.1434x over previous
"""Trainium2 Bass kernel for nn_MultiHeadSelector.

Data-parallel over batch: 32 samples -> 8 cores x 4 samples.
Per sample:
  score = x[:, :, 0, 1:]                       [12, 784]
  per-head top-84 membership -> histogram counts (match_replace trick)
  3x3 [1 2 1; 2 4 2; 1 2 1] conv on 28x28 grid (shifted adds + boundary fixups)
  stable descending argsort via key = cnt*1024 + (1023 - j), ordered top-84
    (11 rounds of DVE max8/max_index/match_replace)
  GCN, using that only row `basic` of g2 is ever read:
    add = lrelu(pos_w[r] @ (relu(pos_w @ (struct@w1)) @ w2))
        = lrelu(((pos_w[r] @ G1) @ w2)          (associativity saves S2)
    G1[n, h] = relu(pos_w @ S1) in natural layout so the row-dot
    contraction runs on PE; pos_wT built by PE block transposes of the
    DVE-accumulated sum over heads.
  hs = hidden_states with row0 += add;  selected = gather(hs, patch_idx).
"""

import os

import numpy as np

import concourse.bass as bass
import concourse.bacc as bacc
import concourse.mybir as mybir
from concourse.tile import TileContext
from concourse.bass_utils import run_bass_kernel_spmd
from concourse.masks import make_identity

F32 = mybir.dt.float32
U32 = mybir.dt.uint32
I32 = mybir.dt.int32

B, C, SP, S, HID = 32, 12, 785, 784, 768
NCORES = 8
BPC = B // NCORES           # samples per core
K = 84
H1 = 512                    # w1 output dim
GRID = 28
NT = 7                      # 784 / 112 partition tiles
PT = 112
NEG = -1.0e30
POS = 1.0e30

LAST_RESULT = None          # BassKernelResults of the most recent run


def _sample_stream(nc, tc, x3, b, pools, consts):
    """Stream x[b] in [128, 785] row tiles (full 16-port DMA), accumulate the
    head sum on DVE, PE-transpose [rows, 112] blocks into pos_wT m-tiles,
    and compute S1 = struct @ (w1/12)."""
    accp, xinp, poswtp, gcnp, psp, smallb = pools
    ident = consts["ident"]
    w1_sb = consts["w1_sb"]
    starts = [1 + 128 * i for i in range(7)]
    sizes = [128] * 6 + [16]

    scoreb = smallb.tile([C, S], F32, tag="scoreb")
    nc.sync.dma_start(scoreb[:], x3[b * C:(b + 1) * C, 0, 1:])
    s1 = [gcnp.tile([PT, H1], F32, tag=f"s1_{t}", name=f"s1_{t}")
          for t in range(NT)]
    for t in range(NT):
        ps = psp.tile([PT, H1], F32, tag="b1", bufs=3, space="PSUM", name="s1ps")
        nc.tensor.matmul(ps[:], lhsT=scoreb[:, t * PT:(t + 1) * PT],
                         rhs=w1_sb[:], start=True, stop=True)
        nc.scalar.copy(s1[t][:], ps[:])

    acc = [accp.tile([sizes[i], SP], F32, tag=f"acc{i}", name=f"acc{i}")
           for i in range(7)]
    poswt = [poswtp.tile([PT, S], F32, tag=f"pw{t}", name=f"pw{t}")
             for t in range(NT)]
    for i in range(7):
        r0, sz = starts[i], sizes[i]
        nc.sync.dma_start(acc[i][:], x3[b * C, r0:r0 + sz, :])
        for c in range(1, C):
            xt = xinp.tile([sizes[i], SP], F32, tag="xt", name="xt")
            nc.sync.dma_start(xt[:sz, :], x3[b * C + c, r0:r0 + sz, :])
            nc.vector.tensor_add(acc[i][:], acc[i][:], xt[:sz, :])
        # transpose [sz, 112] blocks into pos_wT m-tiles
        for mt in range(NT):
            pst = psp.tile([PT, 128], F32, tag="b1", bufs=3, space="PSUM",
                           name="tp")
            nc.tensor.transpose(pst[:, 0:sz],
                                acc[i][:, 1 + mt * PT:1 + (mt + 1) * PT],
                                ident[0:sz, 0:sz])
            nc.scalar.copy(poswt[mt][:, r0 - 1:r0 - 1 + sz], pst[:, 0:sz])
    return s1, poswt


def _sample_gcn(nc, tc, x2d, hid2d, hs_out, b, s1, poswt, pools, consts):
    """G1T[h, n] = relu(S1.T @ pos_wT); row-dot v = pos_w[r] @ G1 done as a
    DVE broadcast-multiply-reduce over G1T; add = lrelu(v @ w2)."""
    accp, xinp, poswtp, gcnp, psp, smallb = pools
    ones12 = consts["ones12"]
    ones1x128 = consts["ones1x128"]
    w2_sb = consts["w2_sb"]
    gidx_u = consts["gidx_u"]

    g1t = [gcnp.tile([128, S], F32, tag=f"g1t_{h}", name=f"g1t_{h}")
           for h in range(4)]
    for h in range(4):
        for c0, c1 in ((0, 512), (512, S)):
            ps = psp.tile([128, 512], F32, tag="b1", bufs=3, space="PSUM",
                          name="g1ps")
            for kt in range(NT):
                nc.tensor.matmul(ps[:, 0:c1 - c0],
                                 lhsT=s1[kt][:, h * 128:(h + 1) * 128],
                                 rhs=poswt[kt][:, c0:c1],
                                 start=(kt == 0), stop=(kt == NT - 1))
            nc.scalar.activation(g1t[h][:, c0:c1], ps[:, 0:c1 - c0],
                                 mybir.ActivationFunctionType.Relu)

    # pos_w row `basic`: gather 12 rows of x, reduce over heads, scale 1/12
    xrow = smallb.tile([C, SP], F32, tag="xrow")
    nc.gpsimd.indirect_dma_start(
        out=xrow[:], out_offset=None, in_=x2d[:],
        in_offset=bass.IndirectOffsetOnAxis(ap=gidx_u[:, b:b + 1], axis=0),
    )
    ps_row = psp.tile([1, S], F32, tag="b2", bufs=2, space="PSUM", name="prowps")
    nc.tensor.matmul(ps_row[:, 0:512], lhsT=ones12[:],
                     rhs=xrow[:, 1:513], start=True, stop=True)
    nc.tensor.matmul(ps_row[:, 512:S], lhsT=ones12[:],
                     rhs=xrow[:, 513:SP], start=True, stop=True)
    prow = smallb.tile([1, S], F32, tag="prow_sb", bufs=1)
    nc.scalar.mul(prow[:], ps_row[:], 1.0 / 12.0)

    # broadcast prow to 128 partitions, then v[h] = sum_n g1t[h]*prow_bc
    psb = psp.tile([128, S], F32, tag="b2", bufs=2, space="PSUM", name="psb")
    nc.tensor.matmul(psb[:, 0:512], lhsT=ones1x128[:], rhs=prow[0:1, 0:512],
                     start=True, stop=True)
    nc.tensor.matmul(psb[:, 512:S], lhsT=ones1x128[:], rhs=prow[0:1, 512:S],
                     start=True, stop=True)
    prow_bc = smallb.tile([128, S], F32, tag="prowbc", bufs=1)
    nc.scalar.copy(prow_bc[:], psb[:])
    vcol = smallb.tile([128, 4], F32, tag="vcol")
    vscr = smallb.tile([128, S], F32, tag="vscr", bufs=1)
    for h in range(4):
        nc.vector.tensor_mul(vscr[:], g1t[h][:], prow_bc[:])
        nc.vector.reduce_sum(vcol[:, h:h + 1], vscr[:],
                             axis=mybir.AxisListType.X)

    # add = lrelu(v @ w2) -> [1, 768]
    ps_add = psp.tile([1, HID], F32, tag="b2", bufs=2, space="PSUM", name="addps")
    for c0, c1 in ((0, 512), (512, HID)):
        for kt in range(4):
            nc.tensor.matmul(ps_add[:, c0:c1], lhsT=vcol[:, kt:kt + 1],
                             rhs=w2_sb[kt][:, c0:c1],
                             start=(kt == 0), stop=(kt == 3))
    # lrelu(v) = max(v, 0.2*v)
    t1 = smallb.tile([1, HID], F32, tag="lr1", bufs=1)
    nc.scalar.copy(t1[:], ps_add[:])
    t2 = smallb.tile([1, HID], F32, tag="lr2", bufs=1)
    nc.vector.tensor_scalar_mul(t2[:], t1[:], 0.2)
    addr = smallb.tile([1, HID], F32, tag="addrow", bufs=1)
    nc.vector.tensor_max(addr[:], t1[:], t2[:])
    hs0 = smallb.tile([1, HID], F32, tag="hs0", bufs=1)
    nc.scalar.dma_start(hs0[:], hid2d[b * SP:b * SP + 1, :])
    nc.vector.tensor_add(hs0[:], hs0[:], addr[:])
    nc.scalar.dma_start(hs_out[b * SP:b * SP + 1, :], hs0[:])


def _phase_a_early(nc, tc, psp, consts):
    """meanscore -> basic_index -> gidx_u (row ids for every sample's pos_w
    row gather). Cheap; emitted early so GCNs are never blocked on it."""
    ident = consts["ident"]
    gbase_sb = consts["gbase_sb"]
    bsel_sb = consts["bsel_sb"]
    gidx_u = consts["gidx_u"]
    score_sb = consts["score_sb"]

    with tc.tile_pool(name="earlyA", bufs=1) as smalla:
        ps_msc = psp.tile([BPC, S], F32, tag="b2", bufs=2, space="PSUM",
                          name="ps_msc")
        for c0, c1 in ((0, 512), (512, S)):
            nc.tensor.matmul(ps_msc[:, c0:c1], lhsT=bsel_sb[:],
                             rhs=score_sb[:, c0:c1], start=True, stop=True)
        meansc = smalla.tile([BPC, S], F32, tag="meansc")
        nc.scalar.copy(meansc[:], ps_msc[:])

        msx = smalla.tile([BPC, 8], F32, tag="msx")
        nc.vector.max(msx[:], meansc[:])
        msi = smalla.tile([BPC, 8], U32, tag="msi")
        nc.vector.max_index(msi[:], msx[:], meansc[:])
        rf = smalla.tile([BPC, 1], F32, tag="rf")
        nc.vector.tensor_copy(rf[:], msi[:, 0:1])
        ps_rT = psp.tile([1, BPC], F32, tag="b1", bufs=3, space="PSUM",
                         name="ps_rT")
        nc.tensor.transpose(ps_rT[:], rf[:], ident[0:BPC, 0:BPC])
        rf_row = smalla.tile([1, BPC], F32, tag="rfrow")
        nc.scalar.copy(rf_row[:], ps_rT[:])
        ps_rbc = psp.tile([C, BPC], F32, tag="b1", bufs=3, space="PSUM",
                          name="ps_rbc")
        nc.tensor.matmul(ps_rbc[:], lhsT=consts["ones1x12"][:], rhs=rf_row[:],
                         start=True, stop=True)
        gidx_f = smalla.tile([C, BPC], F32, tag="gidxf")
        nc.vector.tensor_add(gidx_f[:], gbase_sb[:], ps_rbc[:])
        nc.vector.tensor_copy(gidx_u[:], gidx_f[:])


def _phase_a_topk(nc, tc, psp, consts):
    """Per-head top-84 -> histogram counts (into the const pool). Emitted
    mid-stream to fill DVE slack between accumulation adds."""
    bsel_sb = consts["bsel_sb"]
    score_sb = consts["score_sb"]
    counts = consts["counts_c"]

    with tc.tile_pool(name="sortT", bufs=1) as sortp:
        # per-head ordered top-84 membership via match_replace
        wk = [sortp.tile([BPC * C, S], F32, tag="pp", bufs=2, name=f"wk{i}")
              for i in range(2)]
        nc.vector.tensor_copy(wk[0][:], score_sb[:])
        cur = 0
        for it in range(11):
            mx = sortp.tile([BPC * C, 8], F32, tag="mx8", bufs=2)
            nc.vector.max(mx[:], wk[cur][:])
            if it == 10:
                nc.vector.memset(mx[:, 4:8], POS)
            nc.vector.match_replace(wk[1 - cur][:], mx[:], wk[cur][:], NEG)
            cur = 1 - cur
        eqm = sortp.tile([BPC * C, S], F32, tag="eqm")
        nc.vector.tensor_scalar(eqm[:], wk[cur][:], NEG, None,
                                mybir.AluOpType.is_equal)

        ps_cnt = psp.tile([BPC, S], F32, tag="b2", bufs=2, space="PSUM",
                          name="ps_cnt")
        for c0, c1 in ((0, 512), (512, S)):
            nc.tensor.matmul(ps_cnt[:, c0:c1], lhsT=bsel_sb[:], rhs=eqm[:, c0:c1],
                             start=True, stop=True)
        nc.scalar.copy(counts[:], ps_cnt[:])


def _phase_a_sort(nc, tc, psp, hid2d, sel_out, pidx_out, consts):
    """conv -> stable sort -> patch_idx + selected gather; runs under the
    PE-bound tail."""
    ident = consts["ident"]
    selbase_sb = consts["selbase_sb"]
    iota_rev = consts["iota_rev"]
    counts = consts["counts_c"]

    with (
        tc.tile_pool(name="sortA", bufs=1) as sortp,
        tc.tile_pool(name="smallA", bufs=1) as smalla,
        tc.tile_pool(name="gathA", bufs=1) as gatha,
    ):
        # 3x3 separable conv on the 28x28 grid
        hh = smalla.tile([BPC, S], F32, tag="hh")
        nc.vector.tensor_scalar_mul(hh[:], counts[:], 2.0)
        nc.vector.tensor_add(hh[:, 1:S], hh[:, 1:S], counts[:, 0:S - 1])
        nc.vector.tensor_add(hh[:, 0:S - 1], hh[:, 0:S - 1], counts[:, 1:S])
        # undo wrap-around across 28-col row boundaries (27 positions each)
        hh3a = hh[:, GRID:S].rearrange("p (r c) -> p r c", c=GRID)[:, :, 0:1]
        cn3a = counts[:, GRID - 1:S - 1].rearrange("p (r c) -> p r c", c=GRID)[:, :, 0:1]
        nc.vector.tensor_sub(hh3a, hh3a, cn3a)
        hh3b = hh[:, GRID - 1:S - 1].rearrange("p (r c) -> p r c", c=GRID)[:, :, 0:1]
        cn3b = counts[:, GRID:S].rearrange("p (r c) -> p r c", c=GRID)[:, :, 0:1]
        nc.vector.tensor_sub(hh3b, hh3b, cn3b)
        vv = smalla.tile([BPC, S], F32, tag="vv")
        nc.vector.tensor_scalar_mul(vv[:], hh[:], 2.0)
        nc.vector.tensor_add(vv[:, GRID:S], vv[:, GRID:S], hh[:, 0:S - GRID])
        nc.vector.tensor_add(vv[:, 0:S - GRID], vv[:, 0:S - GRID], hh[:, GRID:S])

        # sort key, ordered top-84
        key = [sortp.tile([BPC * C, S], F32, tag="pp", bufs=2,
                          name=f"key{i}")[0:BPC, :]
               for i in range(2)]
        nc.vector.tensor_scalar_mul(key[0][:], vv[:], 1024.0)
        nc.vector.tensor_add(key[0][:], key[0][:], iota_rev[:])
        mxi = smalla.tile([BPC, 88], U32, tag="mxi")
        cur = 0
        for it in range(11):
            mxk = sortp.tile([BPC, 8], F32, tag="mxk", bufs=2)
            nc.vector.max(mxk[:], key[cur][:])
            nc.vector.max_index(mxi[:, it * 8:it * 8 + 8], mxk[:], key[cur][:])
            if it < 10:
                nc.vector.match_replace(key[1 - cur][:], mxk[:], key[cur][:], NEG)
                cur = 1 - cur

        pidx_f = smalla.tile([BPC, K], F32, tag="pidxf")
        nc.vector.tensor_copy(pidx_f[:], mxi[:, 0:K])
        nc.vector.tensor_scalar_add(pidx_f[:], pidx_f[:], 1.0)
        pidx_i = smalla.tile([BPC, K], I32, tag="pidxi")
        nc.vector.tensor_copy(pidx_i[:], pidx_f[:])
        nc.scalar.dma_start(pidx_out[:], pidx_i[:])

        # selected gather: row ids b*785 + patch_idx, transposed to [84, BPC]
        grow_f = smalla.tile([BPC, K], F32, tag="growf")
        nc.vector.tensor_add(grow_f[:], pidx_f[:],
                             selbase_sb[:, 0:1].to_broadcast([BPC, K]))
        ps_gT = psp.tile([K, BPC], F32, tag="b1", bufs=3, space="PSUM",
                         name="ps_gT")
        nc.tensor.transpose(ps_gT[:], grow_f[:], ident[0:BPC, 0:BPC])
        growT_f = smalla.tile([K, BPC], F32, tag="growtf")
        nc.scalar.copy(growT_f[:], ps_gT[:])
        growT_u = smalla.tile([K, BPC], U32, tag="growtu")
        nc.vector.tensor_copy(growT_u[:], growT_f[:])

        for b in range(BPC):
            g = gatha.tile([K, HID], F32, tag="gath")
            nc.gpsimd.indirect_dma_start(
                out=g[:], out_offset=None, in_=hid2d[:],
                in_offset=bass.IndirectOffsetOnAxis(ap=growT_u[:, b:b + 1], axis=0),
            )
            nc.scalar.dma_start(sel_out[b * K:(b + 1) * K, :], g[:])


def _build_kernel():
    nc = bacc.Bacc("TRN2", target_bir_lowering=False, debug=False,
                   num_devices=NCORES)

    x2d = nc.dram_tensor("x", [BPC * C * SP, SP], F32, kind="ExternalInput").ap()
    hid2d = nc.dram_tensor("hid", [BPC * SP, HID], F32, kind="ExternalInput").ap()
    w1d = nc.dram_tensor("w1", [C, H1], F32, kind="ExternalInput").ap()
    w2d = nc.dram_tensor("w2", [H1, HID], F32, kind="ExternalInput").ap()
    gbased = nc.dram_tensor("gbase", [C, BPC], F32, kind="ExternalInput").ap()
    selbased = nc.dram_tensor("selbase", [BPC, 1], F32, kind="ExternalInput").ap()
    bseld = nc.dram_tensor("bsel", [C * BPC, BPC], F32, kind="ExternalInput").ap()

    hs_out = nc.dram_tensor("hs_out", [BPC * SP, HID], F32, kind="ExternalOutput").ap()
    sel_out = nc.dram_tensor("sel_out", [BPC * K, HID], F32, kind="ExternalOutput").ap()
    pidx_out = nc.dram_tensor("pidx_out", [BPC, K], I32, kind="ExternalOutput").ap()

    x3 = x2d.rearrange("(n s) t -> n s t", s=SP)          # [BPC*C, 785, 785]

    with TileContext(nc) as tc:
        with (
            tc.tile_pool(name="const", bufs=1) as constp,
            tc.tile_pool(name="ps", bufs=1, space="PSUM") as psp,
            tc.tile_pool(name="acc", bufs=2) as accp,
            tc.tile_pool(name="xin", bufs=6) as xinp,
            tc.tile_pool(name="poswt", bufs=2) as poswtp,
            tc.tile_pool(name="gcn", bufs=1) as gcnp,
            tc.tile_pool(name="smallB", bufs=2) as smallb,
        ):
            # ---------------- constants ----------------
            ident = constp.tile([128, 128], F32)
            make_identity(nc, ident[:])
            ones12 = constp.tile([C, 1], F32)
            nc.gpsimd.memset(ones12[:], 1.0)
            ones1x12 = constp.tile([1, C], F32)
            nc.gpsimd.memset(ones1x12[:], 1.0)
            ones1x128 = constp.tile([1, 128], F32)
            nc.gpsimd.memset(ones1x128[:], 1.0)
            iota_rev = constp.tile([BPC, S], F32)
            nc.gpsimd.iota(iota_rev[:], pattern=[[-1, S]], base=1023,
                           channel_multiplier=0,
                           allow_small_or_imprecise_dtypes=True)

            w1_sb = constp.tile([C, H1], F32)
            nc.sync.dma_start(w1_sb[:], w1d[:])
            w2_sb = [constp.tile([128, HID], F32, tag=f"w2_{k}", name=f"w2sb{k}")
                     for k in range(4)]
            for k in range(4):
                nc.scalar.dma_start(w2_sb[k][:], w2d[k * 128:(k + 1) * 128, :])
            gbase_sb = constp.tile([C, BPC], F32)
            nc.scalar.dma_start(gbase_sb[:], gbased[:])
            selbase_sb = constp.tile([BPC, 1], F32)
            nc.scalar.dma_start(selbase_sb[:], selbased[:])
            bsel_sb = constp.tile([C * BPC, BPC], F32)
            nc.scalar.dma_start(bsel_sb[:], bseld[:])
            gidx_u = constp.tile([C, BPC], U32)
            score_sb = constp.tile([BPC * C, S], F32)
            nc.sync.dma_start(score_sb[:], x3[:, 0, 1:])
            counts_c = constp.tile([BPC, S], F32)

            consts = dict(ident=ident, ones12=ones12, ones1x12=ones1x12,
                          ones1x128=ones1x128,
                          iota_rev=iota_rev, gbase_sb=gbase_sb,
                          selbase_sb=selbase_sb, bsel_sb=bsel_sb,
                          gidx_u=gidx_u, w1_sb=w1_sb, w2_sb=w2_sb,
                          score_sb=score_sb, counts_c=counts_c)
            pools = (accp, xinp, poswtp, gcnp, psp, smallb)

            # sample 0 streaming first so DMA/DVE start immediately
            s1_0, poswt_0 = _sample_stream(nc, tc, x3, 0, pools, consts)
            # basic_index/gidx_u only (cheap) — the heavy sort is deferred
            _phase_a_early(nc, tc, psp, consts)
            _sample_gcn(nc, tc, x2d, hid2d, hs_out, 0, s1_0, poswt_0,
                        pools, consts)

            for b in range(1, BPC):
                s1_b, poswt_b = _sample_stream(nc, tc, x3, b, pools, consts)
                if b == 2:
                    # topk half fills DVE slack between accumulation adds
                    _phase_a_topk(nc, tc, psp, consts)
                if b == BPC - 1:
                    # final sort + outputs run under the PE-bound tail
                    _phase_a_sort(nc, tc, psp, hid2d, sel_out, pidx_out,
                                  consts)
                _sample_gcn(nc, tc, x2d, hid2d, hs_out, b, s1_b, poswt_b,
                            pools, consts)

            # bulk hs copy (rows 1..784), DRAM->DRAM on the scalar queue,
            # emitted last so it fills the PE-bound tail
            for b in range(BPC):
                nc.scalar.dma_start(hs_out[b * SP + 1:(b + 1) * SP, :],
                                    hid2d[b * SP + 1:(b + 1) * SP, :])

    nc.compile()
    return nc


def _make_inputs(inputs):
    x = np.ascontiguousarray(inputs["x"], dtype=np.float32)
    hid = np.ascontiguousarray(inputs["hidden_states"], dtype=np.float32)
    # fold the pos_w 1/12 head-mean into w1 (the row-dot path scales prow)
    w1 = np.ascontiguousarray(inputs["w1"], dtype=np.float32) / 12.0
    w2 = np.ascontiguousarray(inputs["w2"], dtype=np.float32)

    gbase = np.empty((C, BPC), np.float32)
    for c in range(C):
        for b in range(BPC):
            gbase[c, b] = (b * C + c) * SP + 1
    selbase = (np.arange(BPC, dtype=np.float32) * SP).reshape(BPC, 1)
    bsel = np.zeros((C * BPC, BPC), np.float32)
    for b in range(BPC):
        bsel[b * C:(b + 1) * C, b] = 1.0

    in_maps = []
    for core in range(NCORES):
        b0 = core * BPC
        in_maps.append({
            "x": x[b0:b0 + BPC].reshape(BPC * C * SP, SP),
            "hid": hid[b0:b0 + BPC].reshape(BPC * SP, HID),
            "w1": w1, "w2": w2,
            "gbase": gbase, "selbase": selbase, "bsel": bsel,
        })
    return in_maps


def _ensure_trace_hooks():
    """If tracing is requested but this container's antenv lacks axon_hooks,
    wire the ctypes NTFF hook from trn_agent_boot and stub the artifact
    upload so run_bass_kernel_spmd's trace path works."""
    import sys
    import types
    try:
        import antenv.axon_hooks  # noqa: F401
        return
    except ImportError:
        pass
    try:
        import antenv
        from trn_agent_boot.trn_boot import _ntff_profile_via_ctypes
        mod = types.ModuleType("antenv.axon_hooks")
        _h = [None]
        mod.set_axon_ntff_profile_hook = lambda h: _h.__setitem__(0, h)
        mod.get_axon_ntff_profile_hook = lambda: _h[0]
        sys.modules["antenv.axon_hooks"] = mod
        antenv.axon_hooks = mod
        hook = _ntff_profile_via_ctypes("/opt/axon/libaxon_pjrt.so")
        if hook is not None:
            mod.set_axon_ntff_profile_hook(hook)
        import concourse.bass_utils as bu
        bu.upload_artifacts = lambda tmpdir: f"file://{tmpdir}"
    except Exception:
        os.environ["BASS_NEVER_TRACE"] = "1"


def kernel(hidden_states, x, contribution, w1, w2):
    global LAST_RESULT
    if os.environ.get("BASS_TRACE"):
        _ensure_trace_hooks()
    nc = _build_kernel()
    in_maps = _make_inputs({"hidden_states": hidden_states, "x": x,
                            "w1": w1, "w2": w2})
    tmpdir = os.environ.get("BASS_KERNEL_TMPDIR")
    if tmpdir:
        os.makedirs(tmpdir, exist_ok=True)
    res = run_bass_kernel_spmd(nc, in_maps, list(range(NCORES)), tmpdir=tmpdir)
    LAST_RESULT = res
    hs = np.concatenate([r["hs_out"].reshape(BPC, SP, HID) for r in res.results])
    sel = np.concatenate([r["sel_out"].reshape(BPC, K, HID) for r in res.results])
    pidx = np.concatenate([r["pidx_out"].reshape(BPC, K) for r in res.results])
    return hs, sel.astype(np.float32), pidx.astype(np.int32)
